# revision 1
# baseline (speedup 1.0000x reference)
"""Trainium2 Bass kernel for IntersectionGNN (3-layer GraphConv, aggr='max').

Strategy:
  out_l = lin_rel(segment_max(x[src], dst)) + lin_root(x) per layer.
  - Host: builds dst-sorted edge order once, computes the max-aggregation per
    layer with np.maximum.reduceat (the gather is data-dependent; no indexed
    DMA primitive is available on this terminal).
  - Device (8 NeuronCores, SPMD): all dense compute. Each core handles one
    (batch, node-half) shard: out = [agg | x] @ [[W_rel],[W_root]] + b_rel
    as a single K=128 matmul per 128-node block, pipelined DMA-in -> PE
    matmul -> DVE bias-add -> DMA-out.
  One program compiled once; 3 launches (one per layer) with per-layer
  weights passed as inputs.
"""
import numpy as np

import concourse.bass as bass
from concourse import mybir
from concourse.bass_utils import run_bass_kernel_spmd

# hardcoded problem shapes
BATCH = 4
N_NODES = 50000
FEAT = 64
N_LAYERS = 3
NCORES = 8

HALF = N_NODES // 2            # 25000 nodes per half-shard
HPAD = 25088                   # padded to 196 blocks of 128
NBLK = HPAD // 128             # 196
NB = 4                         # sbuf tile rotation depth
NPS = 4                        # psum bank rotation depth

_CACHE = {}


def _build_program():
    nc = bass.Bass(num_devices=NCORES)
    catT = nc.declare_dram_parameter("catT", [128, HPAD], mybir.dt.float32, isOutput=False)
    wcat = nc.declare_dram_parameter("wcat", [128, FEAT], mybir.dt.float32, isOutput=False)
    bias = nc.declare_dram_parameter("bias", [128, FEAT], mybir.dt.float32, isOutput=False)
    xo = nc.declare_dram_parameter("xo", [HPAD, FEAT], mybir.dt.float32, isOutput=True)

    import contextlib
    with contextlib.ExitStack() as st:
        block = st.enter_context(nc.Block())
        s_w = st.enter_context(nc.semaphore("s_w"))
        s_in = st.enter_context(nc.semaphore("s_in"))
        s_mm = st.enter_context(nc.semaphore("s_mm"))
        s_bias = st.enter_context(nc.semaphore("s_bias"))
        s_out = st.enter_context(nc.semaphore("s_out"))
        w_t = st.enter_context(nc.sbuf_tensor("w_t", [128, FEAT], mybir.dt.float32))
        b_t = st.enter_context(nc.sbuf_tensor("b_t", [128, FEAT], mybir.dt.float32))
        tin = [st.enter_context(nc.sbuf_tensor(f"tin{k}", [128, 128], mybir.dt.float32))
               for k in range(NB)]
        tout = [st.enter_context(nc.sbuf_tensor(f"tout{k}", [128, FEAT], mybir.dt.float32))
                for k in range(NB)]
        pt = [st.enter_context(nc.psum_tensor(f"pt{k}", [128, FEAT], mybir.dt.float32))
              for k in range(NPS)]

        @block.sync
        def _(sync):
            sync.dma_start(out=w_t[:, :], in_=wcat[:, :]).then_inc(s_w, 16)
            sync.dma_start(out=b_t[:, :], in_=bias[:, :]).then_inc(s_w, 16)
            for i in range(NBLK):
                # WAR on tin slot: PE must have consumed block i-NB
                if i >= NB:
                    sync.wait_ge(s_mm, i - NB + 1)
                sync.dma_start(
                    out=tin[i % NB][:, :],
                    in_=catT[:, i * 128:(i + 1) * 128],
                ).then_inc(s_in, 16)
                # drain an earlier finished output to overlap
                j = i - (NB - 1)
                if j >= 0:
                    sync.wait_ge(s_bias, j + 1)
                    sync.dma_start(
                        out=xo[j * 128:(j + 1) * 128, :],
                        in_=tout[j % NB][:, :],
                    ).then_inc(s_out, 16)
            for j in range(max(0, NBLK - (NB - 1)), NBLK):
                sync.wait_ge(s_bias, j + 1)
                sync.dma_start(
                    out=xo[j * 128:(j + 1) * 128, :],
                    in_=tout[j % NB][:, :],
                ).then_inc(s_out, 16)

        @block.tensor
        def _(tensor):
            tensor.wait_ge(s_w, 16)
            for i in range(NBLK):
                tensor.wait_ge(s_in, 16 * (i + 1))
                if i >= NPS:
                    tensor.wait_ge(s_bias, i - NPS + 1)
                tensor.matmul(
                    pt[i % NPS][:, :], tin[i % NB][:, :], w_t[:, :],
                    start=True, stop=True,
                ).then_inc(s_mm, 1)

        @block.vector
        def _(vector):
            vector.wait_ge(s_w, 32)
            for i in range(NBLK):
                vector.wait_ge(s_mm, i + 1)
                if i >= NB:
                    vector.wait_ge(s_out, 16 * (i - NB + 1))
                vector.tensor_add(
                    tout[i % NB][:, :], pt[i % NPS][:, :], b_t[:, :],
                ).then_inc(s_bias, 1)

    return nc


def _aggregate(x, src_s, starts, empty):
    """segment-max per batch; empty segments -> 0 (PyG scatter-max semantics).

    Per-batch work runs on a thread pool: the gather and reduceat are C loops
    that release the GIL, so the 4 batches run in parallel.
    """
    from concurrent.futures import ThreadPoolExecutor
    n = starts.shape[0]
    agg = np.empty((x.shape[0], n, x.shape[2]), dtype=np.float32)

    def one(b):
        g = x[b][src_s]                      # [E, F] gather in dst-sorted order
        np.maximum.reduceat(g, starts, axis=0, out=agg[b])

    with ThreadPoolExecutor(max_workers=x.shape[0]) as ex:
        list(ex.map(one, range(x.shape[0])))
    agg[:, empty, :] = 0.0
    return agg


def kernel(x, edge_index, W_rel, b_rel, W_root):
    x = np.asarray(x, dtype=np.float32)
    edge_index = np.asarray(edge_index)
    W_rel = np.asarray(W_rel, dtype=np.float32)
    b_rel = np.asarray(b_rel, dtype=np.float32)
    W_root = np.asarray(W_root, dtype=np.float32)

    src = edge_index[0].astype(np.int64)
    dst = edge_index[1].astype(np.int64)

    # dst-sorted edge order + segment starts (computed once per call)
    order = np.argsort(dst, kind="stable")
    src_s = src[order]
    dst_s = dst[order]
    counts = np.bincount(dst_s, minlength=N_NODES)
    empty = counts == 0
    starts = np.zeros(N_NODES, dtype=np.int64)
    starts[1:] = np.cumsum(counts)[:-1]
    starts = np.clip(starts, 0, max(len(dst_s) - 1, 0))

    if "nc" not in _CACHE:
        _CACHE["nc"] = _build_program()
    nc = _CACHE["nc"]

    cur = x
    for l in range(N_LAYERS):
        agg = _aggregate(cur, src_s, starts, empty)
        wcat = np.concatenate([W_rel[l], W_root[l]], axis=0).astype(np.float32)  # [128, 64]
        bias = np.tile(b_rel[l][None, :], (128, 1)).astype(np.float32)           # [128, 64]

        from concurrent.futures import ThreadPoolExecutor

        def mk(c):
            b, h = c // 2, c % 2
            sl = slice(h * HALF, (h + 1) * HALF)
            catT = np.zeros((128, HPAD), dtype=np.float32)
            catT[:FEAT, :HALF] = agg[b, sl].T
            catT[FEAT:, :HALF] = cur[b, sl].T
            return {"catT": catT, "wcat": wcat, "bias": bias}

        with ThreadPoolExecutor(max_workers=NCORES) as ex:
            in_maps = list(ex.map(mk, range(NCORES)))

        import time as _time
        _t0 = _time.time()
        res = run_bass_kernel_spmd(nc, in_maps, list(range(NCORES)))
        _CACHE["launch_s"] = _CACHE.get("launch_s", 0.0) + (_time.time() - _t0)
        nxt = np.empty_like(cur)
        for c in range(NCORES):
            b, h = c // 2, c % 2
            nxt[b, h * HALF:(h + 1) * HALF] = res.results[c]["xo"][:HALF]
        cur = nxt
    return cur



# revision 21
# speedup vs baseline: 6.6529x; 6.6529x over previous
"""Trainium2 Bass kernel for IntersectionGNN (3-layer GraphConv, aggr='max').

out_l = lin_rel(segment_max(x[src], dst)) + lin_root(x) per layer.

Device (8 NeuronCores, SPMD, one (batch, node-half) shard per core): all dense
compute, in transposed form so no on-device or repeated transfers are needed:
    outT = Wcat.T @ [aggT | xT] + b,   Wcat = [W_rel; W_root]  (K=128)
The output is born in [feat, node] layout == the next layer's xT operand, so x
stays resident on device between layers; only aggT (bf16) goes up and outT
(bf16) comes down each layer.  One Bass program, compiled once at import;
launches reuse the same jitted executable (the same PJRT path
bass_utils.run_bass_kernel_spmd takes under axon, hoisted so the trace/compile
is not repeated per call).

Host (single CPU): the segment-max aggregation (no data-dependent gather
primitive exists on this stack: custom GPSIMD ISA fails walrus codegen and
dynamic-offset DMA only honors the first offset on HW).  Nodes are relabeled
by in-degree descending; round r gathers the r-th incoming edge of the first
n_r nodes (a prefix!), accumulated with SIMD np.maximum on an order-preserving
int16 encoding of bf16 — no reduceat, no padding, no dummies.
"""
import contextlib

import numpy as np
import ml_dtypes

import concourse.bass as bass
from concourse import mybir, bass2jax
import jax
from jax.sharding import Mesh, PartitionSpec, NamedSharding

# hardcoded problem shapes
BATCH = 4
N_NODES = 50000
N_EDGES = 800000
FEAT = 64
N_LAYERS = 3
NCORES = 8
HALF = 25088            # padded half-shard (49 groups of 512)
NPOS = 2 * HALF         # 50176 padded positions per batch
GROUPS = HALF // 512    # 49
NB = 4                  # sbuf cat-tile rotation
NOB = 4                 # sbuf out-tile rotation

BF16 = ml_dtypes.bfloat16


def _build_program():
    nc = bass.Bass(num_devices=NCORES)
    aggT = nc.declare_dram_parameter("aggT", [FEAT, HALF], mybir.dt.bfloat16, isOutput=False)
    xT = nc.declare_dram_parameter("xT", [FEAT, HALF], mybir.dt.bfloat16, isOutput=False)
    wcat = nc.declare_dram_parameter("wcat", [2 * FEAT, FEAT], mybir.dt.bfloat16, isOutput=False)
    brep = nc.declare_dram_parameter("brep", [FEAT, 512], mybir.dt.float32, isOutput=False)
    outT = nc.declare_dram_parameter("outT", [FEAT, HALF], mybir.dt.bfloat16, isOutput=True)

    with contextlib.ExitStack() as st:
        block = st.enter_context(nc.Block())
        s_w = st.enter_context(nc.semaphore("s_w"))
        s_in = st.enter_context(nc.semaphore("s_in"))
        s_mm = st.enter_context(nc.semaphore("s_mm"))
        s_ob = st.enter_context(nc.semaphore("s_ob"))
        s_wr = st.enter_context(nc.semaphore("s_wr"))
        w_t = st.enter_context(nc.sbuf_tensor("w_t", [2 * FEAT, FEAT], mybir.dt.bfloat16))
        b_t = st.enter_context(nc.sbuf_tensor("b_t", [FEAT, 512], mybir.dt.float32))
        cat = [st.enter_context(nc.sbuf_tensor(f"cat{k}", [2 * FEAT, 512], mybir.dt.bfloat16))
               for k in range(NB)]
        osb = [st.enter_context(nc.sbuf_tensor(f"osb{k}", [FEAT, 512], mybir.dt.bfloat16))
               for k in range(NOB)]
        ps = [st.enter_context(nc.psum_tensor(f"ps{k}", [FEAT, 512], mybir.dt.float32))
              for k in range(2)]

        @block.sync
        def _(sync):
            sync.dma_start(out=w_t[:, :], in_=wcat[:, :]).then_inc(s_w, 16)
            sync.dma_start(out=b_t[:, :], in_=brep[:, :]).then_inc(s_w, 16)
            for g in range(GROUPS):
                if g >= NB:
                    # WAR: PE must have consumed cat slot g-NB
                    sync.wait_ge(s_mm, g - NB + 1)
                sl = slice(g * 512, (g + 1) * 512)
                sync.dma_start(out=cat[g % NB][0:FEAT, :], in_=aggT[:, sl]).then_inc(s_in, 16)
                sync.dma_start(out=cat[g % NB][FEAT:2 * FEAT, :], in_=xT[:, sl]).then_inc(s_in, 16)

        @block.tensor
        def _(tensor):
            tensor.wait_ge(s_w, 16)
            for g in range(GROUPS):
                tensor.wait_ge(s_in, 32 * (g + 1))
                if g >= 2:
                    tensor.wait_ge(s_ob, g - 1)
                tensor.matmul(
                    ps[g % 2][:, :], w_t[:, :], cat[g % NB][:, :],
                    start=True, stop=True,
                ).then_inc(s_mm, 1)

        @block.vector
        def _(vector):
            vector.wait_ge(s_w, 32)
            for g in range(GROUPS):
                vector.wait_ge(s_mm, g + 1)
                if g >= NOB:
                    vector.wait_ge(s_wr, 16 * (g - NOB + 1))
                vector.tensor_add(
                    osb[g % NOB][:, :], ps[g % 2][:, :], b_t[:, :],
                ).then_inc(s_ob, 1)

        @block.scalar
        def _(scalar):
            for g in range(GROUPS):
                scalar.wait_ge(s_ob, g + 1)
                scalar.dma_start(
                    out=outT[:, g * 512:(g + 1) * 512], in_=osb[g % NOB][:, :],
                ).then_inc(s_wr, 16)

    return nc


# ---------------------------------------------------------------------------
# cached jit launcher (the run_bass_kernel_spmd axon path, hoisted so the
# trace + neuronxcc compile happen once instead of per call)
# ---------------------------------------------------------------------------
_C = {}


def _setup():
    if "fn" in _C:
        return _C
    bass2jax.install_neuronx_cc_hook()
    nc = _build_program()
    partition_name = nc.partition_id_tensor.name if nc.partition_id_tensor else None
    in_names, out_names, out_avals = [], [], []
    for alloc in nc.m.functions[0].allocations:
        if not isinstance(alloc, mybir.MemoryLocationSet):
            continue
        name = alloc.memorylocations[0].name
        if alloc.kind == "ExternalInput":
            if name != partition_name:
                in_names.append(name)
        elif alloc.kind == "ExternalOutput":
            out_names.append(name)
            out_avals.append(jax.core.ShapedArray(tuple(alloc.tensor_shape),
                                                  mybir.dt.np(alloc.dtype)))
    n_params = len(in_names)
    in_names_full = list(in_names) + out_names
    if partition_name is not None:
        in_names_full.append(partition_name)
    donate = tuple(range(n_params, n_params + len(out_names)))

    def _body(*args):
        operands = list(args)
        if partition_name is not None:
            operands.append(bass2jax.partition_id_tensor())
        outs = bass2jax._bass_exec_p.bind(
            *operands,
            out_avals=tuple(out_avals),
            in_names=tuple(in_names_full),
            out_names=tuple(out_names),
            lowering_input_output_aliases=(),
            sim_require_finite=True,
            sim_require_nnan=True,
            nc=nc,
        )
        return tuple(outs)

    devices = jax.devices()[:NCORES]
    mesh = Mesh(np.asarray(devices), ("core",))
    in_specs = (PartitionSpec("core"),) * (n_params + len(out_names))
    out_specs = (PartitionSpec("core"),) * len(out_names)
    from jax.experimental.shard_map import shard_map
    fn = jax.jit(
        shard_map(_body, mesh=mesh, in_specs=in_specs, out_specs=out_specs,
                  check_rep=False),
        donate_argnums=donate, keep_unused=True,
    )
    sh = NamedSharding(mesh, PartitionSpec("core"))
    zmaker = jax.jit(
        lambda: jax.numpy.zeros((NCORES * FEAT, HALF), ml_dtypes.bfloat16),
        out_shardings=sh,
    )
    _C.update(fn=fn, in_names=in_names, devices=devices, mesh=mesh, sh=sh,
              zmaker=zmaker)
    # warm the executable (trace + neuronxcc compile + first dispatch) so
    # kernel() calls only pay transfer + execute
    dummies = []
    for name in in_names:
        if name == "aggT" or name == "xT":
            shape = (NCORES * FEAT, HALF)
            dt = ml_dtypes.bfloat16
        elif name == "wcat":
            shape = (NCORES * 2 * FEAT, FEAT)
            dt = ml_dtypes.bfloat16
        else:  # brep
            shape = (NCORES * FEAT, 512)
            dt = np.float32
        dummies.append(jax.device_put(np.zeros(shape, dt), sh))
    out = fn(*dummies, zmaker())
    jax.block_until_ready(out)
    return _C


_setup()


def _shard_put(arrs):
    """Upload 8 per-core arrays -> one global sharded array (concat axis 0)."""
    C = _C
    bufs = [jax.device_put(a, d) for a, d in zip(arrs, C["devices"])]
    s0, s1 = arrs[0].shape
    return jax.make_array_from_single_device_arrays(
        (NCORES * s0, s1), C["sh"], bufs)


def _mono(u16view):
    """Order-preserving int16 encoding of bf16 bits (involution)."""
    v = u16view.view(np.int16)
    return v ^ ((v >> 15) & np.int16(0x7FFF))


def kernel(x, edge_index, W_rel, b_rel, W_root):
    x = np.asarray(x, dtype=np.float32)
    edge_index = np.asarray(edge_index)
    W_rel = np.asarray(W_rel, dtype=np.float32)
    b_rel = np.asarray(b_rel, dtype=np.float32)
    W_root = np.asarray(W_root, dtype=np.float32)
    C = _setup()
    from concurrent.futures import ThreadPoolExecutor
    up_pool = ThreadPoolExecutor(1)   # serialized background H2D enqueues

    src = edge_index[0].astype(np.int64)
    dst = edge_index[1].astype(np.int64)

    # ---- degree-sorted relabeling (needed before xT0 upload can start) ----
    deg = np.bincount(dst, minlength=N_NODES)
    perm = np.argsort(-deg, kind="stable")
    pos_of = np.empty(N_NODES, dtype=np.int64)
    pos_of[perm] = np.arange(N_NODES)
    deg_sorted = deg[perm]

    def make_T_shard(rows_bf16_b, h):
        """rows [N, F] bf16 (one batch) -> shard [F, HALF] bf16 for half h."""
        lo, hi = h * HALF, min((h + 1) * HALF, rows_bf16_b.shape[0])
        t = np.zeros((FEAT, HALF), dtype=BF16)
        blk = rows_bf16_b[lo:hi]
        t[:, :blk.shape[0]] = np.ascontiguousarray(blk.view(np.uint16).T).view(BF16)
        return t

    # ---- start xT0 + weights upload in background ASAP ----
    xb = x.astype(BF16)
    xbp = xb[:, perm, :]

    def up_initial():
        xs = [make_T_shard(xbp[b], h) for b in range(BATCH) for h in range(2)]
        xT = _shard_put(xs)
        wg, bg = [], []
        for l in range(N_LAYERS):
            w = np.concatenate([W_rel[l], W_root[l]], axis=0).astype(BF16)
            wg.append(_shard_put([w] * NCORES))
            br = np.repeat(b_rel[l].astype(np.float32)[:, None], 512, axis=1)
            bg.append(_shard_put([br] * NCORES))
        return xT, wg, bg

    fut_init = up_pool.submit(up_initial)

    # ---- rest of graph prep (overlaps the xT0 upload) ----
    dp = pos_of[dst]
    sp = pos_of[src].astype(np.int64)
    order = np.argsort(dp, kind="stable")
    dp = dp[order]
    sp = sp[order]
    counts = np.bincount(dp, minlength=N_NODES)
    segstart = np.zeros(N_NODES, dtype=np.int64)
    segstart[1:] = np.cumsum(counts)[:-1]
    k = np.arange(N_EDGES, dtype=np.int64) - segstart[dp]
    maxdeg = int(deg_sorted[0])
    n0 = int(np.count_nonzero(deg))                  # nodes with >=1 edge
    # round r gathers the r-th edge of every position with deg > r (a prefix)
    n_r = np.array([np.searchsorted(-deg_sorted, -(r + 1), side="right")
                    for r in range(maxdeg)], dtype=np.int64)
    roundbase = np.zeros(maxdeg + 1, dtype=np.int64)
    roundbase[1:] = np.cumsum(n_r)
    idx_flat = np.empty(N_EDGES, dtype=np.int64)
    idx_flat[roundbase[k] + dp] = sp

    enc = _mono(np.ascontiguousarray(xbp).view(np.uint16))  # int16 [B,N,F]

    agg16 = np.empty((BATCH, NPOS, FEAT), dtype=np.int16)

    def agg_batch(enc_rows, b):
        """segment-max for one batch via degree-prefix rounds (int16 SIMD)."""
        eb = enc_rows[b].view(np.int64)
        a64 = agg16[b].view(np.int64)
        ab = agg16[b]
        for r in range(maxdeg):
            idx = idx_flat[roundbase[r]:roundbase[r + 1]]
            g = eb[idx]                       # [n_r, F/4] int64 row gather
            if r == 0:
                a64[:g.shape[0]] = g          # plain copy: i64 view fine
            else:
                g16 = g.view(np.int16)        # elementwise max must be i16
                np.maximum(ab[:g16.shape[0]], g16, out=ab[:g16.shape[0]])
        # decode -> bf16 rows; zero empty + pad positions
        dec = _mono(agg16[b]).view(BF16)
        dec[n0:, :] = 0
        return dec

    def agg_shards_for(enc_rows, b):
        dec = agg_batch(enc_rows, b)
        return [make_T_shard(dec, 0), make_T_shard(dec, 1)]

    # ---- layer 0 aggT: per-batch, uploads queued behind xT0 ----
    agg_futs = []
    for b in range(BATCH):
        sh2 = agg_shards_for(enc, b)
        agg_futs.append([up_pool.submit(jax.device_put, sh2[i], C["devices"][2 * b + i])
                         for i in range(2)])

    xT_g, w_gs, b_gs = fut_init.result()

    def gather_aggT(futs):
        bufs = [f.result() for pair in futs for f in pair]
        return jax.make_array_from_single_device_arrays(
            (NCORES * FEAT, HALF), C["sh"], bufs)

    rows = np.empty((BATCH, NPOS, FEAT), dtype=np.uint16)
    for l in range(N_LAYERS):
        aggT_g = gather_aggT(agg_futs)
        zeros = C["zmaker"]()
        named = {"aggT": aggT_g, "xT": xT_g, "wcat": w_gs[l], "brep": b_gs[l]}
        args = [named[n] for n in C["in_names"]]
        (outT_g,) = C["fn"](*args, zeros)
        xT_g = outT_g
        # prefetch all shards, then per-batch: re-encode + next agg + upload
        shards = [s.data for s in outT_g.addressable_shards]
        for s in shards:
            s.copy_to_host_async()
        agg_futs = []
        last = l == N_LAYERS - 1
        if not last:
            enc_next = np.empty((BATCH, NPOS, FEAT), dtype=np.int16)
        for b in range(BATCH):
            for h in range(2):
                sh = np.asarray(shards[2 * b + h])          # [F, HALF] bf16
                rows[b, h * HALF:(h + 1) * HALF] = \
                    np.ascontiguousarray(sh.view(np.uint16).T)
            if not last:
                enc_next[b] = _mono(rows[b])
                sh2 = agg_shards_for(enc_next, b)
                agg_futs.append([up_pool.submit(jax.device_put, sh2[i],
                                                C["devices"][2 * b + i])
                                 for i in range(2)])

    # ---- final assembly: unpermute, cast f32 ----
    up_pool.shutdown(wait=False)
    rows_bf = rows.view(BF16).astype(np.float32)     # [B, NPOS, F]
    out = rows_bf[:, pos_of, :]                      # unpermute -> node order
    return np.ascontiguousarray(out)
